# revision 1
# baseline (speedup 1.0000x reference)
"""Two-layer GCN (PyG GCNConv semantics) on 8 Trainium2 NeuronCores.

Strategy: partition nodes (and their incident edges, by dst) across the 8
cores; each core computes its 12500-node shard of the feature transform,
AllGathers the full transformed table, then aggregates messages for its own
dst nodes with dma_gather (random row gather from the table) + one-hot
selection-matrix matmuls that perform the segment-sum in PSUM.

kernel(**inputs) takes the full unsharded inputs and returns the full
[100000, 64] float32 output.
"""
import sys

sys.path.insert(0, "/opt/trn_rl_repo")

import numpy as np

P = 8          # cores
BLK = 128      # dst nodes per aggregation block (PSUM partition dim)
SBS = 6        # blocks per superblock (one gather call per superblock+chunk)
CHUNK_MAX = 25000  # table rows per gather chunk (int16 index limit 32767)


def _build_partition(src_f, dst_f, norm, n_nodes):
    """Partition/sort/pad edges; build per-core gather+metadata arrays.

    Returns bookkeeping shared by host and device-trace code.

    The table is laid out in block-aligned "quarters": the per-layer
    AllGather is split into one collective per quarter so it can pipeline
    with the transform that produces it and the gathers that consume it.
    Chunk j of the table = concat over cores of their quarter-j rows, so a
    chunk is both (a) one AllGather output tensor and (b) one dma_gather
    source (rows < 32768 so indices fit int16).
    """
    nc_nodes = n_nodes // P
    nb = -(-nc_nodes // BLK)                      # blocks per core
    nsb = -(-nb // SBS)

    nchunk = min(4, nb)                           # quarters (= chunks)
    qb = -(-nb // nchunk)                         # blocks per quarter
    assert qb * BLK * P <= 32768, "chunk rows must fit int16 indices"
    qblocks = [min(qb, nb - j * qb) for j in range(nchunk)]
    qrows = [
        max(0, min(nc_nodes - j * qb * BLK, qblocks[j] * BLK))
        for j in range(nchunk)
    ]

    # per-core sorted edge arrays
    cores = []
    cnt = np.zeros((P, nb, nchunk), np.int64)
    for c in range(P):
        lo, hi = c * nc_nodes, (c + 1) * nc_nodes
        sel = (dst_f >= lo) & (dst_f < hi)
        s = src_f[sel]
        d = dst_f[sel] - lo
        w = norm[sel]
        blk = d // BLK
        scs = s // nc_nodes                       # src core
        sl = s % nc_nodes                         # src local row
        ch = np.minimum(sl // (qb * BLK), nchunk - 1)
        tabidx = scs * np.asarray(qrows)[ch] + (sl - ch * qb * BLK)
        order = np.lexsort((tabidx, ch, blk))
        s, d, w = s[order], d[order], w[order]
        blk, ch, tabidx = blk[order], ch[order], tabidx[order]
        key = blk * nchunk + ch
        cnt[c] = np.bincount(key, minlength=nb * nchunk).reshape(nb, nchunk)
        cores.append((tabidx, d, w))

    L = cnt.max(axis=0)                           # [nb, nchunk] slots per group

    # Call layout: within call (g,k), blocks' slot ranges are packed
    # back-to-back (NOT 128-rounded); only the call total rounds up to 128.
    # A 128-slot tile spanning multiple blocks gets one matmul "unit" per
    # spanned block with a norm-masked one-hot.  All of this bookkeeping is
    # identical across cores (L is a cross-core max), only meta values vary.
    calls = {}
    G = 0     # meta cols (one per unit)
    IC = 0    # idx cols (int16, 16 per 128... num_idxs/16)
    for g in range(nsb):
        bs = list(range(g * SBS, min((g + 1) * SBS, nb)))
        for k in range(nchunk):
            tot = int(sum(L[b, k] for b in bs))
            if tot == 0:
                calls[(g, k)] = None
                continue
            T = -(-tot // BLK)
            num_idxs = T * BLK
            # slot -> block id (-1 for tail padding)
            blk_of = np.full(num_idxs, -1, np.int64)
            s0 = 0
            starts_b = {}
            for b in bs:
                starts_b[b] = s0
                blk_of[s0 : s0 + int(L[b, k])] = b
                s0 += int(L[b, k])
            units = []                             # per tile: [(block, mcol)]
            for t in range(T):
                span = [b for b in bs
                        if starts_b[b] < (t + 1) * BLK
                        and starts_b[b] + int(L[b, k]) > t * BLK]
                if not span:
                    span = [bs[-1]]
                us = []
                for b in span:
                    us.append((b, G))
                    G += 1
                units.append(us)
            calls[(g, k)] = dict(
                T=T, num_idxs=num_idxs, idx_off=IC, units=units,
                blk_of=blk_of, starts_b=starts_b,
            )
            IC += num_idxs // 16

    # per-core arrays
    idx_alls, mdsts, mnrms = [], [], []
    for c in range(P):
        tabidx, d, w = cores[c]
        starts = np.zeros(nb * nchunk + 1, np.int64)
        np.cumsum(cnt[c].reshape(-1), out=starts[1:])
        idx_all = np.zeros((128, IC), np.int16)
        mdst = np.zeros((128, G), np.float32)
        mnrm = np.zeros((128, G), np.float32)
        for g in range(nsb):
            bs = list(range(g * SBS, min((g + 1) * SBS, nb)))
            for k in range(nchunk):
                call = calls[(g, k)]
                if call is None:
                    continue
                ni = call["num_idxs"]
                iv = np.zeros(ni, np.int16)
                dv = np.zeros(ni, np.float32)
                wv = np.zeros(ni, np.float32)
                for b in bs:
                    e0 = starts[b * nchunk + k]
                    n = int(cnt[c, b, k])
                    if n == 0:
                        continue
                    s0 = call["starts_b"][b]
                    iv[s0 : s0 + n] = tabidx[e0 : e0 + n].astype(np.int16)
                    dv[s0 : s0 + n] = (d[e0 : e0 + n] % BLK).astype(np.float32)
                    wv[s0 : s0 + n] = w[e0 : e0 + n]
                blk_of = call["blk_of"]
                for t, us in enumerate(call["units"]):
                    sl = slice(t * BLK, (t + 1) * BLK)
                    for b, mcol in us:
                        mask = blk_of[sl] == b
                        mdst[:, mcol] = np.where(mask, dv[sl], 0.0)
                        mnrm[:, mcol] = np.where(mask, wv[sl], 0.0)
                wrapped = iv.reshape(-1, 16).T     # [16, cols]
                c0 = call["idx_off"]
                idx_all[:, c0 : c0 + wrapped.shape[1]] = np.tile(wrapped, (8, 1))
        idx_alls.append(idx_all)
        mdsts.append(mdst)
        mnrms.append(mnrm)

    return dict(
        nc_nodes=nc_nodes, nb=nb, nchunk=nchunk, qb=qb,
        qblocks=qblocks, qrows=qrows,
        nsb=nsb, calls=calls, G=G, IC=IC,
        idx_alls=idx_alls, mdsts=mdsts, mnrms=mnrms,
    )


def _trace(nc, tile, mybir, bk, in_ch, hid, out_ch, has_b1, has_b2):
    """Emit the Tile program for one core (SPMD: same program, per-core data)."""
    import os

    phases = os.environ.get("GCN_PHASES", "ABCD")
    f32 = mybir.dt.float32
    i16 = mybir.dt.int16
    nc_nodes = bk["nc_nodes"]
    nb, nchunk, nsb = bk["nb"], bk["nchunk"], bk["nsb"]
    npad = nb * BLK
    n_nodes = nc_nodes * P
    RG = [list(range(P))]

    # --- I/O ---
    x_d = nc.dram_tensor("x", [npad, in_ch], f32, kind="ExternalInput")
    xq_d = nc.dram_tensor("xq", [n_nodes, in_ch], f32, kind="ExternalInput")
    w1_d = nc.dram_tensor("w1", [in_ch, hid], f32, kind="ExternalInput")
    w2_d = nc.dram_tensor("w2", [hid, out_ch], f32, kind="ExternalInput")
    iota_d = nc.dram_tensor("iota", [128, 128], f32, kind="ExternalInput")
    iotac_d = nc.dram_tensor("iotac", [128, 1], f32, kind="ExternalInput")
    mdd_d = nc.dram_tensor("mdd", [128, nb], f32, kind="ExternalInput")
    ident_d = nc.dram_tensor("ident", [128, 128], f32, kind="ExternalInput")
    idx_d = nc.dram_tensor("idx", [128, bk["IC"]], i16, kind="ExternalInput")
    mdst_d = nc.dram_tensor("mdst", [128, bk["G"]], f32, kind="ExternalInput")
    mnrm_d = nc.dram_tensor("mnrm", [128, bk["G"]], f32, kind="ExternalInput")
    if has_b1:
        b1_d = nc.dram_tensor("b1bc", [128, hid], f32, kind="ExternalInput")
    if has_b2:
        b2_d = nc.dram_tensor("b2bc", [128, out_ch], f32, kind="ExternalInput")
    out_d = nc.dram_tensor("out", [npad, out_ch], f32, kind="ExternalOutput")

    # --- internal DRAM (quartered so AllGathers pipeline per quarter) ---
    qb, qblocks, qrows = bk["qb"], bk["qblocks"], bk["qrows"]
    OQ = [0]
    for j in range(nchunk):
        OQ.append(OQ[-1] + qrows[j] * P)
    t2q = [
        nc.dram_tensor(f"t2q{j}", [qblocks[j] * BLK, out_ch], f32)
        for j in range(nchunk)
    ]
    table2q = [
        nc.dram_tensor(f"table2q{j}", [qrows[j] * P, out_ch], f32,
                       addr_space="Shared")
        for j in range(nchunk)
    ]

    def pieces(g, segs):
        """Split superblock g's block range into per-quarter DMA pieces.

        segs is a per-quarter tensor list, or a single tensor spanning all
        blocks.  Yields (ap, stage_col0, piece_nblk)."""
        b0 = g * SBS
        nblk = min(SBS, nb - b0)
        if not isinstance(segs, list):
            r0 = b0 * BLK
            yield (
                segs[r0 : r0 + nblk * BLK, :].rearrange(
                    "(nb p) c -> p nb c", p=BLK
                ),
                0, nblk,
            )
            return
        for j in range(nchunk):
            lo = max(b0, j * qb)
            hi = min(b0 + nblk, j * qb + qblocks[j])
            if lo >= hi:
                continue
            r0 = (lo - j * qb) * BLK
            yield (
                segs[j][r0 : r0 + (hi - lo) * BLK, :].rearrange(
                    "(nb p) c -> p nb c", p=BLK
                ),
                lo - b0, hi - lo,
            )

    def nblocks(g):
        return min(SBS, nb - g * SBS)

    with tile.TileContext(nc) as tc:
        with (
            tc.tile_pool(name="const", bufs=1) as cpool,
            tc.tile_pool(name="xload", bufs=2) as xpool,
            tc.tile_pool(name="xt", bufs=4) as xtpool,
            tc.tile_pool(name="hstage", bufs=2) as hpool,
            tc.tile_pool(name="msgs", bufs=5) as mpool,
            tc.tile_pool(name="st", bufs=8) as stpool,
            tc.tile_pool(name="hloc", bufs=6) as hlpool,
        ):
            iota_sb = cpool.tile([128, 128], f32, tag="iota")
            nc.sync.dma_start(iota_sb[:], iota_d[:])
            iotac_sb = cpool.tile([128, 1], f32, tag="iotac")
            nc.sync.dma_start(iotac_sb[:], iotac_d[:])
            mdd_sb = cpool.tile([128, nb], f32, tag="mdd")
            nc.sync.dma_start(mdd_sb[:], mdd_d[:])
            ident_sb = cpool.tile([128, 128], f32, tag="ident")
            nc.sync.dma_start(ident_sb[:], ident_d[:])
            w1_sb = cpool.tile([in_ch, hid], f32, tag="w1")
            nc.sync.dma_start(w1_sb[:], w1_d[:])
            w2_sb = cpool.tile([hid, out_ch], f32, tag="w2")
            nc.sync.dma_start(w2_sb[:], w2_d[:])
            idx_sb = cpool.tile([128, bk["IC"]], i16, tag="idx")
            nc.sync.dma_start(idx_sb[:], idx_d[:])
            mdst_sb = cpool.tile([128, bk["G"]], f32, tag="mdst")
            nc.sync.dma_start(mdst_sb[:], mdst_d[:])
            mnrm_sb = cpool.tile([128, bk["G"]], f32, tag="mnrm")
            nc.sync.dma_start(mnrm_sb[:], mnrm_d[:])
            if has_b1:
                b1_sb = cpool.tile([128, hid], f32, tag="b1")
                nc.sync.dma_start(b1_sb[:], b1_d[:])
            if has_b2:
                b2_sb = cpool.tile([128, out_ch], f32, tag="b2")
                nc.sync.dma_start(b2_sb[:], b2_d[:])

            def transform(src_segs, w_sb, width, dst_segs, psname):
                """dst = src @ W per 128-node block, slab-wise."""
                with tc.tile_pool(name=psname, bufs=4, space="PSUM") as tppool:
                    for g in range(nsb):
                        bs = nblocks(g)
                        xs = xpool.tile([128, bs, in_ch], f32, tag="xs")
                        for ap, col0, nbl in pieces(g, src_segs):
                            nc.sync.dma_start(xs[:, col0 : col0 + nbl, :], ap)
                        hs = hpool.tile([128, bs, hid], f32, tag="hs")
                        for i in range(bs):
                            xt_ps = tppool.tile([128, 128], f32, tag="xt_ps")
                            nc.tensor.transpose(xt_ps[:], xs[:, i, :], ident_sb[:])
                            xt_sb = xtpool.tile([128, 128], f32, tag="xt_sb")
                            nc.scalar.copy(xt_sb[:], xt_ps[:])
                            h_ps = tppool.tile([128, width], f32, tag="h_ps")
                            nc.tensor.matmul(
                                h_ps[:], xt_sb[:], w_sb[:], start=True, stop=True
                            )
                            nc.scalar.copy(hs[:, i, :width], h_ps[:])
                        for ap, col0, nbl in pieces(g, dst_segs):
                            nc.sync.dma_start(
                                ap, hs[:, col0 : col0 + nbl, :width]
                            )

            def emit_ag(tq, tableq, width, emitted, k):
                if emitted[k]:
                    return
                emitted[k] = True
                nc.gpsimd.collective_compute(
                    "AllGather", mybir.AluOpType.bypass, replica_groups=RG,
                    ins=[tq[k][: qrows[k], :]], outs=[tableq[k][:]],
                )

            def aggregate(table_aps, diag_ap, ag_fn, width, epilogue,
                          psname, agg_bufs=8):
                """Chunk-major scatter-sum of norm*table[src] into dst blocks.
                Self-loops enter as a diagonal unit on sequentially-loaded
                local rows."""
                nsb_lim = min(nsb, int(os.environ.get("GCN_NSB_LIMIT", "9999")))
                with tc.tile_pool(
                    name=psname, bufs=agg_bufs, space="PSUM"
                ) as apool:
                    for g in range(nsb_lim):
                        bs = list(range(g * SBS, min((g + 1) * SBS, nb)))
                        nt = {b: 1 for b in bs}        # +1: diagonal unit
                        for k in range(nchunk):
                            call = bk["calls"][(g, k)]
                            if call is None:
                                continue
                            for us in call["units"]:
                                for b, _ in us:
                                    nt[b] += 1
                        pss = {}
                        done = {b: 0 for b in bs}
                        for b in bs:
                            hloc = hlpool.tile([128, width], f32, tag="hloc")
                            nc.sync.dma_start(hloc[:], diag_ap(b))
                            sd = stpool.tile([128, 128], f32, tag="st")
                            nc.vector.tensor_scalar(
                                sd[:], iota_sb[:], iotac_sb[:, 0:1],
                                mdd_sb[:, b : b + 1],
                                op0=mybir.AluOpType.is_equal,
                                op1=mybir.AluOpType.mult,
                            )
                            pss[b] = apool.tile(
                                [128, width], f32, tag="agg", name=f"agg{b}"
                            )
                            nc.tensor.matmul(
                                pss[b][:], sd[:], hloc[:],
                                start=True, stop=(nt[b] == 1),
                            )
                            done[b] = 1
                        for k in range(nchunk):
                            call = bk["calls"][(g, k)]
                            if call is None:
                                continue
                            if ag_fn is not None:
                                ag_fn(k)
                            T = call["T"]
                            m = mpool.tile([128, T, width], f32, tag="msgs")
                            c0 = call["idx_off"]
                            nc.gpsimd.dma_gather(
                                m[:],
                                table_aps[k],
                                idx_sb[:, c0 : c0 + call["num_idxs"] // 16],
                                num_idxs=call["num_idxs"],
                                num_idxs_reg=call["num_idxs"],
                                elem_size=width,
                                single_packet=False,
                            )
                            for t, us in enumerate(call["units"]):
                                for b, mcol in us:
                                    st = stpool.tile([128, 128], f32, tag="st")
                                    nc.vector.tensor_scalar(
                                        st[:], iota_sb[:],
                                        mdst_sb[:, mcol : mcol + 1],
                                        mnrm_sb[:, mcol : mcol + 1],
                                        op0=mybir.AluOpType.is_equal,
                                        op1=mybir.AluOpType.mult,
                                    )
                                    nc.tensor.matmul(
                                        pss[b][:], st[:], m[:, t, :],
                                        start=(done[b] == 0),
                                        stop=(done[b] == nt[b] - 1),
                                    )
                                    done[b] += 1
                        for i, b in enumerate(bs):
                            epilogue(g, i, pss[b])

            # ---- Phase B: layer-1 aggregation of RAW x, then @W1 + ReLU ----
            # A_hat (x@W1) == (A_hat x) @ W1: gather raw x rows (replicated
            # quarter-layout input, no AllGather), transform per block in
            # the epilogue.
            rstage = [None]

            def make_epi1(tp2pool):
                def epi1(g, i, ps):
                    bs = min(SBS, nb - g * SBS)
                    if rstage[0] is None:
                        rstage[0] = hpool.tile(
                            [128, bs, out_ch], f32, tag="hs", name="h2s"
                        )
                    hs = rstage[0]
                    a_sb = xtpool.tile([128, in_ch], f32, tag="a_sb")
                    nc.scalar.copy(a_sb[:], ps[:])
                    at_ps = tp2pool.tile([128, 128], f32, tag="tp", bufs=2)
                    nc.tensor.transpose(at_ps[:], a_sb[:], ident_sb[:])
                    at_sb = xtpool.tile([128, 128], f32, tag="at_sb")
                    nc.scalar.copy(at_sb[:], at_ps[:])
                    h_ps = tp2pool.tile([128, hid], f32, tag="tp", bufs=2)
                    nc.tensor.matmul(
                        h_ps[:], at_sb[:], w1_sb[:], start=True, stop=True
                    )
                    r_sb = xtpool.tile([128, hid], f32, tag="r_sb")
                    if has_b1:
                        nc.vector.tensor_tensor(
                            r_sb[:], h_ps[:], b1_sb[:], mybir.AluOpType.add
                        )
                        nc.scalar.activation(
                            r_sb[:], r_sb[:],
                            mybir.ActivationFunctionType.Relu,
                        )
                    else:
                        nc.scalar.activation(
                            r_sb[:], h_ps[:],
                            mybir.ActivationFunctionType.Relu,
                        )
                    # h2 = relu(...) @ W2 inline (phase C folded in)
                    rt_ps = tp2pool.tile([128, 128], f32, tag="tp", bufs=2)
                    nc.tensor.transpose(rt_ps[:], r_sb[:], ident_sb[:])
                    rt_sb = xtpool.tile([128, 128], f32, tag="at_sb")
                    nc.scalar.copy(rt_sb[:], rt_ps[:])
                    h2_ps = tp2pool.tile([128, out_ch], f32, tag="tp", bufs=2)
                    nc.tensor.matmul(
                        h2_ps[:], rt_sb[:], w2_sb[:], start=True, stop=True
                    )
                    nc.scalar.copy(hs[:, i, :], h2_ps[:])
                    if i == bs - 1:
                        for ap, col0, nbl in pieces(g, t2q):
                            nc.sync.dma_start(ap, hs[:, col0 : col0 + nbl, :])
                        rstage[0] = None

                return epi1

            if "B" in phases:
                xq_aps = [
                    xq_d[OQ[k] : OQ[k] + qrows[k] * P, :] for k in range(nchunk)
                ]
                with tc.tile_pool(name="tpB", bufs=1, space="PSUM") as tp2pool:
                    for _r in range(int(os.environ.get("GCN_REPEAT", "1"))):
                        aggregate(
                            xq_aps,
                            lambda b: x_d[b * BLK : (b + 1) * BLK, :],
                            None, in_ch, make_epi1(tp2pool), f"aggB{_r}",
                            agg_bufs=6,
                        )

            # ---- Phase C folded into phase B's epilogue ----
            ag2_emitted = [False] * nchunk

            # ---- Phase D: layer-2 aggregation -> out ----
            ostage = [None]

            def epi2(g, i, ps):
                bs = min(SBS, nb - g * SBS)
                if ostage[0] is None:
                    ostage[0] = hpool.tile(
                        [128, bs, out_ch], f32, tag="os", name="o2s"
                    )
                os_ = ostage[0]
                if has_b2:
                    nc.vector.tensor_tensor(
                        os_[:, i, :], ps[:], b2_sb[:], mybir.AluOpType.add
                    )
                else:
                    nc.scalar.copy(os_[:, i, :], ps[:])
                if i == bs - 1:
                    for ap, col0, nbl in pieces(g, out_d):
                        nc.sync.dma_start(ap, os_[:, col0 : col0 + nbl, :])
                    ostage[0] = None

            if "D" in phases:
                aggregate(
                    [table2q[k][:] for k in range(nchunk)],
                    lambda b: t2q[b // qb][
                        (b - (b // qb) * qb) * BLK
                        : (b - (b // qb) * qb + 1) * BLK, :
                    ],
                    lambda k: emit_ag(t2q, table2q, out_ch, ag2_emitted, k),
                    out_ch, epi2, "aggD",
                )


def _prepare(x, edge_index, W1, b1, W2, b2):
    """Host preprocessing + trace + compile. Returns (nc, bk, in_maps)."""
    import concourse.bacc as bacc
    import concourse.mybir as mybir
    from concourse import tile

    x = np.asarray(x, dtype=np.float32)
    edge_index = np.asarray(edge_index)
    W1 = np.asarray(W1, dtype=np.float32)
    b1 = np.asarray(b1, dtype=np.float32)
    W2 = np.asarray(W2, dtype=np.float32)
    b2 = np.asarray(b2, dtype=np.float32)

    n_nodes, in_ch = x.shape
    hid = W1.shape[1]
    out_ch = W2.shape[1]
    assert in_ch == 128 and hid == 128, "transform path assumes 128 channels"

    # --- graph preprocessing (index arithmetic only) ---
    src = edge_index[0].astype(np.int64)
    dst = edge_index[1].astype(np.int64)
    loops = np.arange(n_nodes, dtype=np.int64)
    src_f = np.concatenate([src, loops])
    dst_f = np.concatenate([dst, loops])
    deg = np.bincount(dst_f, minlength=n_nodes).astype(np.float32)
    dinv = np.where(deg > 0, 1.0 / np.sqrt(deg), 0.0).astype(np.float32)
    # self-loops are handled as per-block diagonal units (sequential loads),
    # not as gathered edges
    norm_e = (dinv[src] * dinv[dst]).astype(np.float32)

    bk = _build_partition(src, dst, norm_e, n_nodes)
    nc_nodes, nb = bk["nc_nodes"], bk["nb"]
    npad = nb * BLK

    has_b1 = bool(np.any(b1))
    has_b2 = bool(np.any(b2))

    nc = bacc.Bacc(
        "TRN2", target_bir_lowering=False, debug=False, num_devices=P,
        dynamic_dma_scratch_size=32768,
    )
    _trace(nc, tile, mybir, bk, in_ch, hid, out_ch, has_b1, has_b2)
    nc.compile()

    iota_np = np.tile(np.arange(128, dtype=np.float32), (128, 1))
    iotac_np = np.arange(128, dtype=np.float32)[:, None]
    ident_np = np.eye(128, dtype=np.float32)

    # xq: x re-laid-out in (quarter, core, local) order — the gather-table
    # layout — replicated to every core (no layer-1 AllGather needed).
    qb, qrows = bk["qb"], bk["qrows"]
    nchunk = bk["nchunk"]
    xq = np.empty((n_nodes, in_ch), np.float32)
    off = 0
    for j in range(nchunk):
        for c in range(P):
            r0 = c * nc_nodes + j * qb * BLK
            xq[off : off + qrows[j]] = x[r0 : r0 + qrows[j]]
            off += qrows[j]
    assert off == n_nodes

    in_maps = []
    for c in range(P):
        xs = np.zeros((npad, in_ch), np.float32)
        xs[:nc_nodes] = x[c * nc_nodes : (c + 1) * nc_nodes]
        dd = np.zeros(npad, np.float32)
        dd[:nc_nodes] = dinv[c * nc_nodes : (c + 1) * nc_nodes] ** 2
        m = dict(
            x=xs, xq=xq, w1=W1, w2=W2, iota=iota_np, iotac=iotac_np,
            ident=ident_np,
            idx=bk["idx_alls"][c], mdst=bk["mdsts"][c], mnrm=bk["mnrms"][c],
            mdd=dd.reshape(nb, BLK).T.copy(),
        )
        if has_b1:
            m["b1bc"] = np.tile(b1[None, :], (128, 1)).astype(np.float32)
        if has_b2:
            m["b2bc"] = np.tile(b2[None, :], (128, 1)).astype(np.float32)
        in_maps.append(m)

    return nc, bk, in_maps


def kernel(x, edge_index, W1, b1, W2, b2):
    from concourse.bass_utils import run_bass_kernel_spmd

    nc, bk, in_maps = _prepare(x, edge_index, W1, b1, W2, b2)
    res = run_bass_kernel_spmd(nc, in_maps, core_ids=list(range(P)))
    out = np.concatenate(
        [res.results[c]["out"][: bk["nc_nodes"]] for c in range(P)], axis=0
    )
    return out.astype(np.float32)

